# revision 13
# baseline (speedup 1.0000x reference)
"""v5: multi-run dynamic-DMA gathers (25 instr/step, 4 SWDGE queues).

Device is layout-agnostic; walkers wander along a fixed permutation PI
(gather run order -> bounced re-chunk order). Host tracks labels A_t,
supplies u/masks per step in the wandering layout, and decodes outputs.
Runs m=136,272,... of each instruction are corrupted by HW (first desc of
SDMA engines 1-15); slots whose 8-step orbit hits them carry pad walkers.
"""

import os

import numpy as np

import concourse.bacc as bacc
import concourse.mybir as mybir
import concourse.tile as tile
from concourse.bass_utils import run_bass_kernel_spmd

N = 100000
E = 3_200_000
WALKS_PER_NODE = 4
L_TOTAL = 8
S = N * WALKS_PER_NODE
NCORES = 8
SPC = S // NCORES  # 50000 walkers per core
P = 128
K = 18  # gather instructions per step
CM = 23  # offset-tile columns per instruction
M = P * CM  # 2176 runs per instruction
C2 = K * CM  # 425 slot columns
SLOTS = P * C2  # 54400
SENT_PAD = 128
DEG_SHIFT = 17
NQ = 4

I32 = mybir.dt.int32
F32 = mybir.dt.float32

LAST_EXEC_TIME_NS = None
LAST_RESULTS = None


def _gather_q(gp, out_ap, in_ap, offset_ap, queue):
    out_l = gp.lower_ap_dma(out_ap, for_indirect_dma=True)
    in_l = gp.lower_ap_dma(in_ap, for_indirect_dma=True)
    off_l = gp.lower_ap_dma(offset_ap)
    coef = 1
    for i in range(1, len(in_ap.shape)):
        coef *= in_ap.shape[i]
    in_l[0].dynamic_ap_info = mybir.DynamicAccessPatternInfo(
        c=0,
        actual_ap=out_ap.ap,
        indirect_dim_max_index=in_ap.shape[0],
        offset_expr=[
            mybir.DynamicAccessPatternOffsetExpr(
                coef=coef,
                aff_expr=mybir.DynamicAccessPatternOffsetExprAffExpr(
                    kind="IndirectArgId", arg_id=1
                ),
            )
        ],
    )
    in_l.append(off_l[0])
    return gp.add_instruction(
        mybir.InstDMACopy(
            name=gp.bass.get_next_instruction_name(),
            queue=queue,
            mode="Copy",
            ins=in_l,
            outs=out_l,
            oob_is_err=True,
            cce_op=mybir.AluOpType.bypass,
        )
    )


def _build_nc():
    nc = bacc.Bacc(
        "TRN2", target_bir_lowering=False, debug=False, num_devices=NCORES,
        num_swdge_queues=NQ,
    )
    table = nc.dram_tensor("table", [E + SENT_PAD, 2], I32, kind="ExternalInput")
    r0 = nc.dram_tensor("r0", [P, C2], I32, kind="ExternalInput")
    d0 = nc.dram_tensor("d0", [P, C2], I32, kind="ExternalInput")
    u8 = nc.dram_tensor("u8", [L_TOTAL, P, C2], F32, kind="ExternalInput")
    msk = nc.dram_tensor("msk", [L_TOTAL, P, C2], I32, kind="ExternalInput")
    walks = nc.dram_tensor("walks", [L_TOTAL - 1, P, C2], I32, kind="ExternalOutput")
    stage = nc.dram_tensor("stage", [K, M * 2], I32, kind="Internal")

    op = mybir.AluOpType
    MASKC = (1 << DEG_SHIFT) - 1
    qctr = 0
    with tile.TileContext(nc) as tc:
        with tc.tile_pool(name="sbuf", bufs=1) as pool:
            u_sb = pool.tile([P, L_TOTAL, C2], F32)
            m_sb = pool.tile([P, L_TOTAL, C2], I32)
            r_cur = pool.tile([P, C2], I32, tag="rcur")
            d_cur = pool.tile([P, C2], I32, tag="dcur")
            walks_sb = pool.tile([P, L_TOTAL - 1, C2], I32)

            nc.sync.dma_start(u_sb[:], u8.ap().rearrange("t p c -> p t c"))
            nc.sync.dma_start(m_sb[:], msk.ap().rearrange("t p c -> p t c"))
            nc.sync.dma_start(r_cur[:], r0.ap())
            nc.sync.dma_start(d_cur[:], d0.ap())

            state_prev = None
            for t in range(L_TOTAL):
                if t >= 1:
                    w0 = state_prev[:].rearrange("p (c r) -> p c r", r=2)[:, :, 0:1]
                    w1 = state_prev[:].rearrange("p (c r) -> p c r", r=2)[:, :, 1:2]
                    if t >= 2:
                        nc.vector.tensor_scalar(
                            walks_sb[:, t - 2, :], w0, MASKC, None, op.bitwise_and
                        )
                    nc.vector.tensor_scalar(
                        d_cur[:], w0, DEG_SHIFT, None, op.logical_shift_right
                    )
                    nc.vector.tensor_copy(r_cur[:], w1)
                # sanitize non-real slots: d = d*(1-m)+m ; r = r*(1-m)
                im = pool.tile([P, C2], I32, tag="im", bufs=2)
                nc.vector.tensor_scalar(
                    im[:], m_sb[:, t, :], -1, None, op.mult
                )
                nc.vector.tensor_scalar(im[:], im[:], 1, None, op.add)  # 1-m
                nc.vector.tensor_tensor(d_cur[:], d_cur[:], im[:], op=op.mult)
                nc.vector.tensor_tensor(
                    d_cur[:], d_cur[:], m_sb[:, t, :], op=op.add
                )
                nc.vector.tensor_tensor(r_cur[:], r_cur[:], im[:], op=op.mult)

                df = pool.tile([P, C2], F32, tag="df", bufs=2)
                prod = pool.tile([P, C2], F32, tag="prod", bufs=2)
                oi = pool.tile([P, C2], I32, tag="oi", bufs=2)
                of = pool.tile([P, C2], F32, tag="of", bufs=2)
                gt = pool.tile([P, C2], I32, tag="gt", bufs=2)
                off = pool.tile([P, C2], I32, tag="off", bufs=2)
                dm = pool.tile([P, C2], I32, tag="dm", bufs=2)

                nc.vector.tensor_copy(df[:], d_cur[:])
                nc.vector.tensor_tensor(prod[:], u_sb[:, t, :], df[:], op=op.mult)
                nc.vector.tensor_copy(oi[:], prod[:])
                nc.vector.tensor_copy(of[:], oi[:])
                nc.vector.tensor_tensor(gt[:], of[:], prod[:], op=op.is_gt)
                nc.vector.tensor_tensor(off[:], oi[:], gt[:], op=op.subtract)
                nc.vector.tensor_scalar(dm[:], d_cur[:], -1, 0, op.add, op.max)
                nc.vector.tensor_tensor(off[:], off[:], dm[:], op=op.min)

                g25 = pool.tile([K, M * 2], I32, tag="g25", bufs=2)
                for k in range(K):
                    sl = slice(k * CM, (k + 1) * CM)
                    ek = pool.tile([P, CM], I32, tag=f"ek{k}", bufs=2)
                    nc.vector.tensor_tensor(
                        ek[:], r_cur[:, sl], off[:, sl], op=op.add
                    )
                    qname = f"qPoolDynamic{(qctr % NQ) or ''}"
                    qctr += 1
                    dest = g25[k : k + 1, :].rearrange("a (m r) -> a m r", r=2)
                    _gather_q(nc.gpsimd, dest, table.ap(), ek[:], qname)

                # bounce: rows -> DRAM flat -> [128, C2*2]
                nc.sync.dma_start(stage.ap(), g25[:])
                state = pool.tile([P, C2 * 2], I32, tag="state", bufs=2)
                nc.sync.dma_start(
                    state[:], stage.ap().rearrange("a b -> (a b)").rearrange(
                        "(p x) -> p x", p=P
                    ),
                )
                state_prev = state

            w0 = state_prev[:].rearrange("p (c r) -> p c r", r=2)[:, :, 0:1]
            nc.vector.tensor_scalar(
                walks_sb[:, L_TOTAL - 2, :], w0, MASKC, None, op.bitwise_and
            )
            nc.sync.dma_start(walks.ap().rearrange("t p c -> p t c"), walks_sb[:])

    nc.compile()
    return nc


def _host_layout():
    """PI over linear slots s = q*C2 + c, plus bad-run mask per step."""
    q, c = np.meshgrid(np.arange(P), np.arange(C2), indexing="ij")
    k = c // CM
    j = c % CM
    m = j * P + q
    flat = (k * M + m).reshape(-1)  # pi(s) as linear slot id
    badpre = ((m % (M // 16) == 0) & (m > 0)).reshape(-1)
    return flat, badpre


def kernel(edge_index, edge_weight, rand_unif):
    global LAST_EXEC_TIME_NS, LAST_RESULTS
    edge_index = np.asarray(edge_index, dtype=np.int32)
    edge_weight = np.asarray(edge_weight, dtype=np.float32)
    rand_unif = np.asarray(rand_unif, dtype=np.float32)

    row, col = edge_index[0], edge_index[1]
    order = np.argsort(row, kind="stable")
    col_s = col[order].astype(np.int32)
    deg = np.bincount(row, minlength=N).astype(np.int32)
    rowptr = np.zeros(N, dtype=np.int32)
    np.cumsum(deg[:-1], out=rowptr[1:])
    assert int(deg.max()) < 128

    iso = np.flatnonzero(deg == 0).astype(np.int32)
    assert len(iso) <= SENT_PAD
    rowptr2 = rowptr.copy()
    rowptr2[iso] = E + np.arange(len(iso), dtype=np.int32)

    table = np.zeros((E + SENT_PAD, 2), dtype=np.int32)
    table[:E, 0] = col_s + (deg[col_s] << DEG_SHIFT)
    table[:E, 1] = rowptr2[col_s]
    if len(iso):
        table[E : E + len(iso), 0] = iso
        table[E : E + len(iso), 1] = E + np.arange(len(iso), dtype=np.int32)

    PI, badpre = _host_layout()
    # slots whose orbit avoids bad runs for all 8 steps
    ok = ~badpre.copy()
    cur = np.arange(SLOTS)
    for _ in range(1, L_TOTAL):
        cur = PI[cur]
        ok &= ~badpre[cur]
    good = np.flatnonzero(ok)
    assert len(good) >= SPC, f"only {len(good)} good slots"

    start_all = np.tile(np.arange(N, dtype=np.int32), WALKS_PER_NODE)

    in_maps = []
    decode = []  # per core: list of A_t label arrays (t = 1..8)
    for cc in range(NCORES):
        starts = start_all[cc * SPC : (cc + 1) * SPC]
        A = np.full(SLOTS, -1, dtype=np.int64)
        A[good[:SPC]] = np.arange(SPC)
        r0 = np.zeros(SLOTS, dtype=np.int32)
        d0 = np.ones(SLOTS, dtype=np.int32)
        live = A >= 0
        r0[live] = rowptr2[starts[A[live]]]
        d0[live] = deg[starts[A[live]]]
        u = np.zeros((L_TOTAL, SLOTS), dtype=np.float32)
        msk = np.zeros((L_TOTAL, SLOTS), dtype=np.int32)
        As = []
        At = A
        ur = rand_unif[cc * SPC : (cc + 1) * SPC]
        for t in range(L_TOTAL):
            live = At >= 0
            u[t, live] = ur[At[live], t]
            msk[t, ~live] = 1
            Anext = np.full(SLOTS, -1, dtype=np.int64)
            Anext[PI] = At
            As.append(Anext)
            At = Anext
        decode.append(As)
        in_maps.append(
            {
                "table": table,
                "r0": r0.reshape(P, C2),
                "d0": d0.reshape(P, C2),
                "u8": np.ascontiguousarray(u.reshape(L_TOTAL, P, C2)),
                "msk": np.ascontiguousarray(msk.reshape(L_TOTAL, P, C2)),
            }
        )

    nc = _build_nc()
    trace = bool(int(os.environ.get("KERNEL_TRACE", "0")))
    if trace:
        try:
            import profhook

            profhook.install()
        except Exception:
            trace = False
    res = run_bass_kernel_spmd(nc, in_maps, core_ids=list(range(NCORES)), trace=trace)
    LAST_EXEC_TIME_NS = res.exec_time_ns
    LAST_RESULTS = res

    targets = np.empty((S, L_TOTAL - 1), dtype=np.int32)
    for cc in range(NCORES):
        w = res.results[cc]["walks"].reshape(L_TOTAL - 1, SLOTS)
        for t in range(1, L_TOTAL):
            At1 = decode[cc][t]  # labels after gather t = As[t] (A_{t+1})
            live = At1 >= 0
            plane = w[t - 1]
            targets[cc * SPC + At1[live], t - 1] = plane[live]

    roots = np.repeat(start_all[:, None], L_TOTAL - 1, axis=1)
    added = np.stack([roots.reshape(-1), targets.reshape(-1)])
    out_edge_index = np.concatenate([edge_index, added], axis=1)
    out_edge_weight = np.concatenate(
        [edge_weight, np.ones(added.shape[1], dtype=np.float32)]
    )
    return out_edge_index, out_edge_weight


# revision 14
# speedup vs baseline: 1.1664x; 1.1664x over previous
"""v5: multi-run dynamic-DMA gathers (25 instr/step, 4 SWDGE queues).

Device is layout-agnostic; walkers wander along a fixed permutation PI
(gather run order -> bounced re-chunk order). Host tracks labels A_t,
supplies u/masks per step in the wandering layout, and decodes outputs.
Runs m=136,272,... of each instruction are corrupted by HW (first desc of
SDMA engines 1-15); slots whose 8-step orbit hits them carry pad walkers.
"""

import os

import numpy as np

import concourse.bacc as bacc
import concourse.mybir as mybir
import concourse.tile as tile
from concourse.bass_utils import run_bass_kernel_spmd

N = 100000
E = 3_200_000
WALKS_PER_NODE = 4
L_TOTAL = 8
S = N * WALKS_PER_NODE
NCORES = 8
SPC = S // NCORES  # 50000 walkers per core
P = 128
K = 34  # gather instructions per step
CM = 13  # offset-tile columns per instruction
M = P * CM  # 2176 runs per instruction
C2 = K * CM  # 425 slot columns
SLOTS = P * C2  # 54400
SENT_PAD = 128
DEG_SHIFT = 17
NQ = 4

I32 = mybir.dt.int32
F32 = mybir.dt.float32

LAST_EXEC_TIME_NS = None
LAST_RESULTS = None


def _gather_q(gp, out_ap, in_ap, offset_ap, queue):
    out_l = gp.lower_ap_dma(out_ap, for_indirect_dma=True)
    in_l = gp.lower_ap_dma(in_ap, for_indirect_dma=True)
    off_l = gp.lower_ap_dma(offset_ap)
    coef = 1
    for i in range(1, len(in_ap.shape)):
        coef *= in_ap.shape[i]
    in_l[0].dynamic_ap_info = mybir.DynamicAccessPatternInfo(
        c=0,
        actual_ap=out_ap.ap,
        indirect_dim_max_index=in_ap.shape[0],
        offset_expr=[
            mybir.DynamicAccessPatternOffsetExpr(
                coef=coef,
                aff_expr=mybir.DynamicAccessPatternOffsetExprAffExpr(
                    kind="IndirectArgId", arg_id=1
                ),
            )
        ],
    )
    in_l.append(off_l[0])
    return gp.add_instruction(
        mybir.InstDMACopy(
            name=gp.bass.get_next_instruction_name(),
            queue=queue,
            mode="Copy",
            ins=in_l,
            outs=out_l,
            oob_is_err=True,
            cce_op=mybir.AluOpType.bypass,
        )
    )


def _build_nc():
    nc = bacc.Bacc(
        "TRN2", target_bir_lowering=False, debug=False, num_devices=NCORES,
        num_swdge_queues=NQ,
    )
    table = nc.dram_tensor("table", [E + SENT_PAD, 2], I32, kind="ExternalInput")
    r0 = nc.dram_tensor("r0", [P, C2], I32, kind="ExternalInput")
    d0 = nc.dram_tensor("d0", [P, C2], I32, kind="ExternalInput")
    u8 = nc.dram_tensor("u8", [L_TOTAL, P, C2], F32, kind="ExternalInput")
    msk = nc.dram_tensor("msk", [L_TOTAL, P, C2], I32, kind="ExternalInput")
    walks = nc.dram_tensor("walks", [L_TOTAL - 1, P, C2], I32, kind="ExternalOutput")
    stage = nc.dram_tensor("stage", [K, M * 2], I32, kind="Internal")

    op = mybir.AluOpType
    MASKC = (1 << DEG_SHIFT) - 1
    qctr = 0
    with tile.TileContext(nc) as tc:
        with tc.tile_pool(name="sbuf", bufs=1) as pool:
            u_sb = pool.tile([P, L_TOTAL, C2], F32)
            m_sb = pool.tile([P, L_TOTAL, C2], I32)
            r_cur = pool.tile([P, C2], I32, tag="rcur")
            d_cur = pool.tile([P, C2], I32, tag="dcur")
            walks_sb = pool.tile([P, L_TOTAL - 1, C2], I32)

            nc.sync.dma_start(u_sb[:], u8.ap().rearrange("t p c -> p t c"))
            nc.sync.dma_start(m_sb[:], msk.ap().rearrange("t p c -> p t c"))
            nc.sync.dma_start(r_cur[:], r0.ap())
            nc.sync.dma_start(d_cur[:], d0.ap())

            state_prev = None
            for t in range(L_TOTAL):
                if t >= 1:
                    w0 = state_prev[:].rearrange("p (c r) -> p c r", r=2)[:, :, 0:1]
                    w1 = state_prev[:].rearrange("p (c r) -> p c r", r=2)[:, :, 1:2]
                    if t >= 2:
                        nc.vector.tensor_scalar(
                            walks_sb[:, t - 2, :], w0, MASKC, None, op.bitwise_and
                        )
                    nc.vector.tensor_scalar(
                        d_cur[:], w0, DEG_SHIFT, None, op.logical_shift_right
                    )
                    nc.vector.tensor_copy(r_cur[:], w1)
                # sanitize non-real slots: d = d*(1-m)+m ; r = r*(1-m)
                im = pool.tile([P, C2], I32, tag="im", bufs=2)
                nc.vector.tensor_scalar(
                    im[:], m_sb[:, t, :], -1, None, op.mult
                )
                nc.vector.tensor_scalar(im[:], im[:], 1, None, op.add)  # 1-m
                nc.vector.tensor_tensor(d_cur[:], d_cur[:], im[:], op=op.mult)
                nc.vector.tensor_tensor(
                    d_cur[:], d_cur[:], m_sb[:, t, :], op=op.add
                )
                nc.vector.tensor_tensor(r_cur[:], r_cur[:], im[:], op=op.mult)

                df = pool.tile([P, C2], F32, tag="df", bufs=2)
                prod = pool.tile([P, C2], F32, tag="prod", bufs=2)
                oi = pool.tile([P, C2], I32, tag="oi", bufs=2)
                of = pool.tile([P, C2], F32, tag="of", bufs=2)
                gt = pool.tile([P, C2], I32, tag="gt", bufs=2)
                off = pool.tile([P, C2], I32, tag="off", bufs=2)
                dm = pool.tile([P, C2], I32, tag="dm", bufs=2)

                nc.vector.tensor_copy(df[:], d_cur[:])
                nc.vector.tensor_tensor(prod[:], u_sb[:, t, :], df[:], op=op.mult)
                nc.vector.tensor_copy(oi[:], prod[:])
                nc.vector.tensor_copy(of[:], oi[:])
                nc.vector.tensor_tensor(gt[:], of[:], prod[:], op=op.is_gt)
                nc.vector.tensor_tensor(off[:], oi[:], gt[:], op=op.subtract)
                nc.vector.tensor_scalar(dm[:], d_cur[:], -1, 0, op.add, op.max)
                nc.vector.tensor_tensor(off[:], off[:], dm[:], op=op.min)

                g25 = pool.tile([K, M * 2], I32, tag="g25", bufs=2)
                for k in range(K):
                    sl = slice(k * CM, (k + 1) * CM)
                    ek = pool.tile([P, CM], I32, tag=f"ek{k}", bufs=2)
                    nc.vector.tensor_tensor(
                        ek[:], r_cur[:, sl], off[:, sl], op=op.add
                    )
                    qname = f"qPoolDynamic{(qctr % NQ) or ''}"
                    qctr += 1
                    dest = g25[k : k + 1, :].rearrange("a (m r) -> a m r", r=2)
                    _gather_q(nc.gpsimd, dest, table.ap(), ek[:], qname)

                # bounce: rows -> DRAM flat -> [128, C2*2]
                nc.sync.dma_start(stage.ap(), g25[:])
                state = pool.tile([P, C2 * 2], I32, tag="state", bufs=2)
                nc.sync.dma_start(
                    state[:], stage.ap().rearrange("a b -> (a b)").rearrange(
                        "(p x) -> p x", p=P
                    ),
                )
                state_prev = state

            w0 = state_prev[:].rearrange("p (c r) -> p c r", r=2)[:, :, 0:1]
            nc.vector.tensor_scalar(
                walks_sb[:, L_TOTAL - 2, :], w0, MASKC, None, op.bitwise_and
            )
            nc.sync.dma_start(walks.ap().rearrange("t p c -> p t c"), walks_sb[:])

    nc.compile()
    return nc


def _host_layout():
    """PI over linear slots s = q*C2 + c, plus bad-run mask per step."""
    q, c = np.meshgrid(np.arange(P), np.arange(C2), indexing="ij")
    k = c // CM
    j = c % CM
    m = j * P + q
    flat = (k * M + m).reshape(-1)  # pi(s) as linear slot id
    badpre = ((m % (M // 16) == 0) & (m > 0)).reshape(-1)
    return flat, badpre


def kernel(edge_index, edge_weight, rand_unif):
    global LAST_EXEC_TIME_NS, LAST_RESULTS
    edge_index = np.asarray(edge_index, dtype=np.int32)
    edge_weight = np.asarray(edge_weight, dtype=np.float32)
    rand_unif = np.asarray(rand_unif, dtype=np.float32)

    row, col = edge_index[0], edge_index[1]
    order = np.argsort(row, kind="stable")
    col_s = col[order].astype(np.int32)
    deg = np.bincount(row, minlength=N).astype(np.int32)
    rowptr = np.zeros(N, dtype=np.int32)
    np.cumsum(deg[:-1], out=rowptr[1:])
    assert int(deg.max()) < 128

    iso = np.flatnonzero(deg == 0).astype(np.int32)
    assert len(iso) <= SENT_PAD
    rowptr2 = rowptr.copy()
    rowptr2[iso] = E + np.arange(len(iso), dtype=np.int32)

    table = np.zeros((E + SENT_PAD, 2), dtype=np.int32)
    table[:E, 0] = col_s + (deg[col_s] << DEG_SHIFT)
    table[:E, 1] = rowptr2[col_s]
    if len(iso):
        table[E : E + len(iso), 0] = iso
        table[E : E + len(iso), 1] = E + np.arange(len(iso), dtype=np.int32)

    PI, badpre = _host_layout()
    # slots whose orbit avoids bad runs for all 8 steps
    ok = ~badpre.copy()
    cur = np.arange(SLOTS)
    for _ in range(1, L_TOTAL):
        cur = PI[cur]
        ok &= ~badpre[cur]
    good = np.flatnonzero(ok)
    assert len(good) >= SPC, f"only {len(good)} good slots"

    start_all = np.tile(np.arange(N, dtype=np.int32), WALKS_PER_NODE)

    in_maps = []
    decode = []  # per core: list of A_t label arrays (t = 1..8)
    for cc in range(NCORES):
        starts = start_all[cc * SPC : (cc + 1) * SPC]
        A = np.full(SLOTS, -1, dtype=np.int64)
        A[good[:SPC]] = np.arange(SPC)
        r0 = np.zeros(SLOTS, dtype=np.int32)
        d0 = np.ones(SLOTS, dtype=np.int32)
        live = A >= 0
        r0[live] = rowptr2[starts[A[live]]]
        d0[live] = deg[starts[A[live]]]
        u = np.zeros((L_TOTAL, SLOTS), dtype=np.float32)
        msk = np.zeros((L_TOTAL, SLOTS), dtype=np.int32)
        As = []
        At = A
        ur = rand_unif[cc * SPC : (cc + 1) * SPC]
        for t in range(L_TOTAL):
            live = At >= 0
            u[t, live] = ur[At[live], t]
            msk[t, ~live] = 1
            Anext = np.full(SLOTS, -1, dtype=np.int64)
            Anext[PI] = At
            As.append(Anext)
            At = Anext
        decode.append(As)
        in_maps.append(
            {
                "table": table,
                "r0": r0.reshape(P, C2),
                "d0": d0.reshape(P, C2),
                "u8": np.ascontiguousarray(u.reshape(L_TOTAL, P, C2)),
                "msk": np.ascontiguousarray(msk.reshape(L_TOTAL, P, C2)),
            }
        )

    nc = _build_nc()
    trace = bool(int(os.environ.get("KERNEL_TRACE", "0")))
    if trace:
        try:
            import profhook

            profhook.install()
        except Exception:
            trace = False
    res = run_bass_kernel_spmd(nc, in_maps, core_ids=list(range(NCORES)), trace=trace)
    LAST_EXEC_TIME_NS = res.exec_time_ns
    LAST_RESULTS = res

    targets = np.empty((S, L_TOTAL - 1), dtype=np.int32)
    for cc in range(NCORES):
        w = res.results[cc]["walks"].reshape(L_TOTAL - 1, SLOTS)
        for t in range(1, L_TOTAL):
            At1 = decode[cc][t]  # labels after gather t = As[t] (A_{t+1})
            live = At1 >= 0
            plane = w[t - 1]
            targets[cc * SPC + At1[live], t - 1] = plane[live]

    roots = np.repeat(start_all[:, None], L_TOTAL - 1, axis=1)
    added = np.stack([roots.reshape(-1), targets.reshape(-1)])
    out_edge_index = np.concatenate([edge_index, added], axis=1)
    out_edge_weight = np.concatenate(
        [edge_weight, np.ones(added.shape[1], dtype=np.float32)]
    )
    return out_edge_index, out_edge_weight


# revision 15
# speedup vs baseline: 1.2512x; 1.0727x over previous
"""v5: multi-run dynamic-DMA gathers (25 instr/step, 4 SWDGE queues).

Device is layout-agnostic; walkers wander along a fixed permutation PI
(gather run order -> bounced re-chunk order). Host tracks labels A_t,
supplies u/masks per step in the wandering layout, and decodes outputs.
Runs m=136,272,... of each instruction are corrupted by HW (first desc of
SDMA engines 1-15); slots whose 8-step orbit hits them carry pad walkers.
"""

import os

import numpy as np

import concourse.bacc as bacc
import concourse.mybir as mybir
import concourse.tile as tile
from concourse.bass_utils import run_bass_kernel_spmd

N = 100000
E = 3_200_000
WALKS_PER_NODE = 4
L_TOTAL = 8
S = N * WALKS_PER_NODE
NCORES = 8
SPC = S // NCORES  # 50000 walkers per core
P = 128
K = 25  # gather instructions per step
CM = 17  # offset-tile columns per instruction
M = P * CM  # 2176 runs per instruction
C2 = K * CM  # 425 slot columns
SLOTS = P * C2  # 54400
SENT_PAD = 128
DEG_SHIFT = 17
NQ = 4

I32 = mybir.dt.int32
F32 = mybir.dt.float32

LAST_EXEC_TIME_NS = None
LAST_RESULTS = None


def _gather_q(gp, out_ap, in_ap, offset_ap, queue):
    out_l = gp.lower_ap_dma(out_ap, for_indirect_dma=True)
    in_l = gp.lower_ap_dma(in_ap, for_indirect_dma=True)
    off_l = gp.lower_ap_dma(offset_ap)
    coef = 1
    for i in range(1, len(in_ap.shape)):
        coef *= in_ap.shape[i]
    in_l[0].dynamic_ap_info = mybir.DynamicAccessPatternInfo(
        c=0,
        actual_ap=out_ap.ap,
        indirect_dim_max_index=in_ap.shape[0],
        offset_expr=[
            mybir.DynamicAccessPatternOffsetExpr(
                coef=coef,
                aff_expr=mybir.DynamicAccessPatternOffsetExprAffExpr(
                    kind="IndirectArgId", arg_id=1
                ),
            )
        ],
    )
    in_l.append(off_l[0])
    return gp.add_instruction(
        mybir.InstDMACopy(
            name=gp.bass.get_next_instruction_name(),
            queue=queue,
            mode="Copy",
            ins=in_l,
            outs=out_l,
            oob_is_err=True,
            cce_op=mybir.AluOpType.bypass,
        )
    )


def _build_nc():
    nc = bacc.Bacc(
        "TRN2", target_bir_lowering=False, debug=False, num_devices=NCORES,
        num_swdge_queues=NQ,
    )
    table = nc.dram_tensor("table", [E + SENT_PAD, 2], I32, kind="ExternalInput")
    r0 = nc.dram_tensor("r0", [P, C2], I32, kind="ExternalInput")
    d0 = nc.dram_tensor("d0", [P, C2], I32, kind="ExternalInput")
    u8 = nc.dram_tensor("u8", [L_TOTAL, P, C2], F32, kind="ExternalInput")
    msk = nc.dram_tensor("msk", [L_TOTAL, P, C2], I32, kind="ExternalInput")
    walks = nc.dram_tensor("walks", [L_TOTAL - 1, P, C2], I32, kind="ExternalOutput")
    stage = nc.dram_tensor("stage", [K, M * 2], I32, kind="Internal")

    op = mybir.AluOpType
    MASKC = (1 << DEG_SHIFT) - 1
    qctr = 0
    with tile.TileContext(nc) as tc:
        with tc.tile_pool(name="sbuf", bufs=1) as pool:
            u_sb = pool.tile([P, L_TOTAL, C2], F32)
            m_sb = pool.tile([P, L_TOTAL, C2], I32)
            r_cur = pool.tile([P, C2], I32, tag="rcur")
            d_cur = pool.tile([P, C2], I32, tag="dcur")
            walks_sb = pool.tile([P, L_TOTAL - 1, C2], I32)

            nc.sync.dma_start(u_sb[:], u8.ap().rearrange("t p c -> p t c"))
            nc.sync.dma_start(m_sb[:], msk.ap().rearrange("t p c -> p t c"))
            nc.sync.dma_start(r_cur[:], r0.ap())
            nc.sync.dma_start(d_cur[:], d0.ap())

            state_prev = None
            for t in range(L_TOTAL):
                if t >= 1:
                    w0 = state_prev[:].rearrange("p (c r) -> p c r", r=2)[:, :, 0:1]
                    w1 = state_prev[:].rearrange("p (c r) -> p c r", r=2)[:, :, 1:2]
                    if t >= 2:
                        nc.vector.tensor_scalar(
                            walks_sb[:, t - 2, :], w0, MASKC, None, op.bitwise_and
                        )
                    nc.vector.tensor_scalar(
                        d_cur[:], w0, DEG_SHIFT, None, op.logical_shift_right
                    )
                    nc.vector.tensor_copy(r_cur[:], w1)
                # sanitize non-real slots: d = d*(1-m)+m ; r = r*(1-m)
                im = pool.tile([P, C2], I32, tag="im", bufs=2)
                nc.vector.tensor_scalar(
                    im[:], m_sb[:, t, :], -1, None, op.mult
                )
                nc.vector.tensor_scalar(im[:], im[:], 1, None, op.add)  # 1-m
                nc.vector.tensor_tensor(d_cur[:], d_cur[:], im[:], op=op.mult)
                nc.vector.tensor_tensor(
                    d_cur[:], d_cur[:], m_sb[:, t, :], op=op.add
                )
                nc.vector.tensor_tensor(r_cur[:], r_cur[:], im[:], op=op.mult)

                df = pool.tile([P, C2], F32, tag="df", bufs=2)
                prod = pool.tile([P, C2], F32, tag="prod", bufs=2)
                oi = pool.tile([P, C2], I32, tag="oi", bufs=2)
                of = pool.tile([P, C2], F32, tag="of", bufs=2)
                gt = pool.tile([P, C2], I32, tag="gt", bufs=2)
                off = pool.tile([P, C2], I32, tag="off", bufs=2)
                dm = pool.tile([P, C2], I32, tag="dm", bufs=2)

                nc.vector.tensor_copy(df[:], d_cur[:])
                nc.vector.tensor_tensor(prod[:], u_sb[:, t, :], df[:], op=op.mult)
                nc.vector.tensor_copy(oi[:], prod[:])
                nc.vector.tensor_copy(of[:], oi[:])
                nc.vector.tensor_tensor(gt[:], of[:], prod[:], op=op.is_gt)
                nc.vector.tensor_tensor(off[:], oi[:], gt[:], op=op.subtract)
                nc.vector.tensor_scalar(dm[:], d_cur[:], -1, 0, op.add, op.max)
                nc.vector.tensor_tensor(off[:], off[:], dm[:], op=op.min)

                g25 = pool.tile([K, M * 2], I32, tag="g25", bufs=2)
                for k in range(K):
                    sl = slice(k * CM, (k + 1) * CM)
                    ek = pool.tile([P, CM], I32, tag=f"ek{k}", bufs=2)
                    nc.vector.tensor_tensor(
                        ek[:], r_cur[:, sl], off[:, sl], op=op.add
                    )
                    qname = f"qPoolDynamic{(qctr % NQ) or ''}"
                    qctr += 1
                    dest = g25[k : k + 1, :].rearrange("a (m r) -> a m r", r=2)
                    _gather_q(nc.gpsimd, dest, table.ap(), ek[:], qname)

                # bounce: rows -> DRAM flat -> [128, C2*2]
                nc.sync.dma_start(stage.ap(), g25[:])
                state = pool.tile([P, C2 * 2], I32, tag="state", bufs=2)
                nc.sync.dma_start(
                    state[:], stage.ap().rearrange("a b -> (a b)").rearrange(
                        "(p x) -> p x", p=P
                    ),
                )
                state_prev = state

            w0 = state_prev[:].rearrange("p (c r) -> p c r", r=2)[:, :, 0:1]
            nc.vector.tensor_scalar(
                walks_sb[:, L_TOTAL - 2, :], w0, MASKC, None, op.bitwise_and
            )
            nc.sync.dma_start(walks.ap().rearrange("t p c -> p t c"), walks_sb[:])

    nc.compile()
    return nc


def _host_layout():
    """PI over linear slots s = q*C2 + c, plus bad-run mask per step."""
    q, c = np.meshgrid(np.arange(P), np.arange(C2), indexing="ij")
    k = c // CM
    j = c % CM
    m = j * P + q
    flat = (k * M + m).reshape(-1)  # pi(s) as linear slot id
    badpre = ((m % (M // 16) == 0) & (m > 0)).reshape(-1)
    return flat, badpre


def kernel(edge_index, edge_weight, rand_unif):
    global LAST_EXEC_TIME_NS, LAST_RESULTS
    edge_index = np.asarray(edge_index, dtype=np.int32)
    edge_weight = np.asarray(edge_weight, dtype=np.float32)
    rand_unif = np.asarray(rand_unif, dtype=np.float32)

    row, col = edge_index[0], edge_index[1]
    order = np.argsort(row, kind="stable")
    col_s = col[order].astype(np.int32)
    deg = np.bincount(row, minlength=N).astype(np.int32)
    rowptr = np.zeros(N, dtype=np.int32)
    np.cumsum(deg[:-1], out=rowptr[1:])
    assert int(deg.max()) < 128

    iso = np.flatnonzero(deg == 0).astype(np.int32)
    assert len(iso) <= SENT_PAD
    rowptr2 = rowptr.copy()
    rowptr2[iso] = E + np.arange(len(iso), dtype=np.int32)

    table = np.zeros((E + SENT_PAD, 2), dtype=np.int32)
    table[:E, 0] = col_s + (deg[col_s] << DEG_SHIFT)
    table[:E, 1] = rowptr2[col_s]
    if len(iso):
        table[E : E + len(iso), 0] = iso
        table[E : E + len(iso), 1] = E + np.arange(len(iso), dtype=np.int32)

    PI, badpre = _host_layout()
    # slots whose orbit avoids bad runs for all 8 steps
    ok = ~badpre.copy()
    cur = np.arange(SLOTS)
    for _ in range(1, L_TOTAL):
        cur = PI[cur]
        ok &= ~badpre[cur]
    good = np.flatnonzero(ok)
    assert len(good) >= SPC, f"only {len(good)} good slots"

    start_all = np.tile(np.arange(N, dtype=np.int32), WALKS_PER_NODE)

    in_maps = []
    decode = []  # per core: list of A_t label arrays (t = 1..8)
    for cc in range(NCORES):
        starts = start_all[cc * SPC : (cc + 1) * SPC]
        A = np.full(SLOTS, -1, dtype=np.int64)
        A[good[:SPC]] = np.arange(SPC)
        r0 = np.zeros(SLOTS, dtype=np.int32)
        d0 = np.ones(SLOTS, dtype=np.int32)
        live = A >= 0
        r0[live] = rowptr2[starts[A[live]]]
        d0[live] = deg[starts[A[live]]]
        u = np.zeros((L_TOTAL, SLOTS), dtype=np.float32)
        msk = np.zeros((L_TOTAL, SLOTS), dtype=np.int32)
        As = []
        At = A
        ur = rand_unif[cc * SPC : (cc + 1) * SPC]
        for t in range(L_TOTAL):
            live = At >= 0
            u[t, live] = ur[At[live], t]
            msk[t, ~live] = 1
            Anext = np.full(SLOTS, -1, dtype=np.int64)
            Anext[PI] = At
            As.append(Anext)
            At = Anext
        decode.append(As)
        in_maps.append(
            {
                "table": table,
                "r0": r0.reshape(P, C2),
                "d0": d0.reshape(P, C2),
                "u8": np.ascontiguousarray(u.reshape(L_TOTAL, P, C2)),
                "msk": np.ascontiguousarray(msk.reshape(L_TOTAL, P, C2)),
            }
        )

    nc = _build_nc()
    trace = bool(int(os.environ.get("KERNEL_TRACE", "0")))
    if trace:
        try:
            import profhook

            profhook.install()
        except Exception:
            trace = False
    res = run_bass_kernel_spmd(nc, in_maps, core_ids=list(range(NCORES)), trace=trace)
    LAST_EXEC_TIME_NS = res.exec_time_ns
    LAST_RESULTS = res

    targets = np.empty((S, L_TOTAL - 1), dtype=np.int32)
    for cc in range(NCORES):
        w = res.results[cc]["walks"].reshape(L_TOTAL - 1, SLOTS)
        for t in range(1, L_TOTAL):
            At1 = decode[cc][t]  # labels after gather t = As[t] (A_{t+1})
            live = At1 >= 0
            plane = w[t - 1]
            targets[cc * SPC + At1[live], t - 1] = plane[live]

    roots = np.repeat(start_all[:, None], L_TOTAL - 1, axis=1)
    added = np.stack([roots.reshape(-1), targets.reshape(-1)])
    out_edge_index = np.concatenate([edge_index, added], axis=1)
    out_edge_weight = np.concatenate(
        [edge_weight, np.ones(added.shape[1], dtype=np.float32)]
    )
    return out_edge_index, out_edge_weight
